# revision 5
# baseline (speedup 1.0000x reference)
"""Masked multi-head attention (B=8, N=1024, C=512, H=8) on 8 TRN2 NeuronCores.

Strategy: pure data parallel — core b computes batch element b. All compute is
done in "feature-major" (transposed) layout so no on-chip transposes are needed:

  xT [C, N] --(qkv_w.T tiles as lhsT)--> qkT [1024, N]  (q,k features x tokens)
  v computed token-major [N, 512] directly (lhsT = xT tiles)
  scores^T[k, q] = kT_tile.T @ qT  (contraction over head dim, 64 partitions)
  p^T = exp(scale * s^T)  (ACT, no max subtraction: |scale*s| < 3)
  p^T *= keep^T  (bernoulli keep mask, DVE, bf16)
  out^T[d, q] (+ rowsum in row 64) = sum_k v_aug.T @ p^T  (ones column trick)
  out^T normalized by broadcasted 1/rowsum, + v-bias (folded via bv x rowsum)
  y^T[o, q] = proj_w.T tiles as lhsT @ a^T ; host transposes y^T back.

The bernoulli mask is reproduced on host with jax.random (same PRNG the
reference uses) and streamed as a bf16 0/1 keep tensor (the dominant memory
traffic — this problem is memory-bound by design).
"""

import numpy as np

B, N, C, H = 8, 1024, 512, 8
HD = C // H  # 64
SCALE = HD**-0.5
NT = N // 128  # 8 token tiles
CT = C // 128  # 4 feature tiles

_cache = {}


def _bf16():
    import ml_dtypes

    return ml_dtypes.bfloat16


def _build_nc():
    from concourse import bacc, bass, mybir, tile

    f32 = mybir.dt.float32
    bf16 = mybir.dt.bfloat16

    nc = bacc.Bacc("TRN2", target_bir_lowering=False, debug=False, num_devices=8)

    xT_d = nc.declare_dram_parameter("xT", [C, N], bf16, isOutput=False)
    qkvw_d = nc.declare_dram_parameter("qkvw", [C, 3 * C], bf16, isOutput=False)
    projw_d = nc.declare_dram_parameter("projw", [C, C], bf16, isOutput=False)
    keep_d = nc.declare_dram_parameter("keepT", [H, N, N], bf16, isOutput=False)
    qkb_d = nc.declare_dram_parameter("qkb", [128, 8], f32, isOutput=False)
    bv_d = nc.declare_dram_parameter("bv", [1, 8 * 66], f32, isOutput=False)
    pjb_d = nc.declare_dram_parameter("pjb", [128, 4], f32, isOutput=False)
    out_d = nc.declare_dram_parameter("out", [C, N], f32, isOutput=True)

    with tile.TileContext(nc) as tc:
        with (
            tc.tile_pool(name="const", bufs=1) as cpool,
            tc.tile_pool(name="qk", bufs=1) as qkpool,
            tc.tile_pool(name="vp", bufs=1) as vpool,
            tc.tile_pool(name="ap", bufs=1) as apool,
            tc.tile_pool(name="yp", bufs=1) as ypool,
            tc.tile_pool(name="keep", bufs=4) as kpool,
            tc.tile_pool(name="pp", bufs=2) as ppool,
            tc.tile_pool(name="rs", bufs=2) as rspool,
            tc.tile_pool(name="rb", bufs=2) as rbpool,
            tc.tile_pool(name="ps", bufs=4, space="PSUM") as pspool,
        ):
            # ---- load constants / inputs ----
            xt = cpool.tile([128, CT, N], bf16)
            nc.sync.dma_start(xt[:], xT_d[:].rearrange("(t p) n -> p t n", p=128))
            qkvw = cpool.tile([128, CT, 3 * C], bf16)
            nc.sync.dma_start(qkvw[:], qkvw_d[:].rearrange("(t p) n -> p t n", p=128))
            projw = cpool.tile([128, CT, C], bf16)
            nc.sync.dma_start(projw[:], projw_d[:].rearrange("(t p) n -> p t n", p=128))
            qkb = cpool.tile([128, 8], f32)
            nc.sync.dma_start(qkb[:], qkb_d[:])
            bv = cpool.tile([1, 8 * 66], f32)
            nc.sync.dma_start(bv[:], bv_d[:])
            pjb = cpool.tile([128, 4], f32)
            nc.sync.dma_start(pjb[:], pjb_d[:])

            # ---- QKV projections ----
            # q,k feature-major: qkT[feat, tok]; m-tile m covers feats m*128..+128
            qkT = qkpool.tile([128, 8, N], bf16)
            for m in range(8):
                ps = pspool.tile([128, N], f32, tag="ps")
                for t in range(CT):
                    lhsT = qkvw[:, t, m * 128 : (m + 1) * 128]
                    for n2 in range(2):
                        nc.tensor.matmul(
                            ps[:, n2 * 512 : (n2 + 1) * 512],
                            lhsT,
                            xt[:, t, n2 * 512 : (n2 + 1) * 512],
                            start=(t == 0),
                            stop=(t == CT - 1),
                        )
                nc.vector.tensor_scalar_add(qkT[:, m, :], ps[:], qkb[:, m : m + 1])

            # v token-major with ones column: v[tok, head, 66]
            # col 0..63 = v dims, col 64 = 1.0 (rowsum), col 65 = 0.0 (pad)
            vsb = vpool.tile([128, NT, H, 66], bf16)
            nc.gpsimd.memset(vsb[:, :, :, 64:65], 1.0)
            nc.gpsimd.memset(vsb[:, :, :, 65:66], 0.0)
            for mt in range(NT):
                psv = pspool.tile([128, N], f32, tag="ps")
                for t in range(CT):
                    nc.tensor.matmul(
                        psv[:, 0:512],
                        xt[:, t, mt * 128 : (mt + 1) * 128],
                        qkvw[:, t, 2 * C : 3 * C],
                        start=(t == 0),
                        stop=(t == CT - 1),
                    )
                nc.vector.tensor_copy(
                    vsb[:, mt, :, 0:64],
                    psv[:, 0:512].rearrange("p (h d) -> p h d", d=64),
                )

            # ---- attention per head ----
            aT = apool.tile([128, CT, N], bf16)
            for h in range(H):
                pbase = (h % 2) * 64
                qT_h = qkT[pbase : pbase + 64, h // 2, :]
                kT_h = qkT[pbase : pbase + 64, 4 + h // 2, :]

                pT = ppool.tile([128, NT, N], bf16, tag="pT")
                for kt in range(NT):
                    ps_s = pspool.tile([128, N], f32, tag="ps")
                    for n2 in range(2):
                        nc.tensor.matmul(
                            ps_s[:, n2 * 512 : (n2 + 1) * 512],
                            kT_h[:, kt * 128 : (kt + 1) * 128],
                            qT_h[:, n2 * 512 : (n2 + 1) * 512],
                            start=True,
                            stop=True,
                        )
                    nc.scalar.activation(
                        pT[:, kt, :],
                        ps_s[:],
                        mybir.ActivationFunctionType.Exp,
                        bias=0.0,
                        scale=float(SCALE),
                    )
                    keep_t = kpool.tile([128, N], bf16, tag="keep")
                    nc.sync.dma_start(
                        keep_t[:], keep_d[h, kt * 128 : (kt + 1) * 128, :]
                    )
                    nc.vector.tensor_tensor(
                        pT[:, kt, :], pT[:, kt, :], keep_t[:], mybir.AluOpType.mult
                    )

                # PV: out^T[d, q] rows 0..63, rowsum row 64
                ps_o = pspool.tile([128, N], f32, tag="ps")
                for kt in range(NT):
                    for n2 in range(2):
                        nc.tensor.matmul(
                            ps_o[0:66, n2 * 512 : (n2 + 1) * 512],
                            vsb[:, kt, h, :],
                            pT[:, kt, n2 * 512 : (n2 + 1) * 512],
                            start=(kt == 0),
                            stop=False,
                        )
                rs = rspool.tile([1, N], f32, tag="rs")
                nc.vector.tensor_copy(rs[0:1, :], ps_o[64:65, :])
                # += bv (x) rowsum  (folds the v bias: (pv + rs*bv)/rs = pv/rs + bv)
                for n2 in range(2):
                    nc.tensor.matmul(
                        ps_o[0:66, n2 * 512 : (n2 + 1) * 512],
                        bv[0:1, h * 66 : (h + 1) * 66],
                        rs[0:1, n2 * 512 : (n2 + 1) * 512],
                        start=False,
                        stop=(n2 == 1),
                    )
                recip = rspool.tile([1, N], f32, tag="recip")
                nc.vector.reciprocal_approx_fast(recip[0:1, :], rs[0:1, :])
                rb = rbpool.tile([64, N], f32, tag="rb")
                nc.gpsimd.partition_broadcast(rb[:], recip[0:1, :])
                nc.vector.tensor_tensor(
                    aT[pbase : pbase + 64, h // 2, :],
                    ps_o[0:64, :],
                    rb[:],
                    mybir.AluOpType.mult,
                )

            # ---- output projection ----
            yT = ypool.tile([128, CT, N], f32)
            for mo in range(CT):
                ps_y = pspool.tile([128, N], f32, tag="ps")
                for t in range(CT):
                    lhsT = projw[:, t, mo * 128 : (mo + 1) * 128]
                    for n2 in range(2):
                        nc.tensor.matmul(
                            ps_y[:, n2 * 512 : (n2 + 1) * 512],
                            lhsT,
                            aT[:, t, n2 * 512 : (n2 + 1) * 512],
                            start=(t == 0),
                            stop=(t == CT - 1),
                        )
                nc.scalar.activation(
                    yT[:, mo, :],
                    ps_y[:],
                    mybir.ActivationFunctionType.Identity,
                    bias=pjb[:, mo : mo + 1],
                    scale=1.0,
                )
            nc.sync.dma_start(out_d[:].rearrange("(t p) n -> p t n", p=128), yT[:])

    nc.compile()
    return nc


def _get_nc():
    if "nc" not in _cache:
        _cache["nc"] = _build_nc()
    return _cache["nc"]


def _get_keepT():
    """keepT[b, h, k, q] = 1 - mask[b, h, q, k], bf16, one array per batch."""
    if "keepT" not in _cache:
        import jax

        mask = np.asarray(
            jax.random.bernoulli(jax.random.key(42), 0.5, (B, H, N, N))
        )
        keep = (~mask).astype(_bf16())
        keepT = np.ascontiguousarray(keep.transpose(0, 1, 3, 2))
        _cache["keepT"] = keepT
    return _cache["keepT"]


def kernel(x, qkv_w, qkv_b, proj_w, proj_b):
    from concourse.bass_utils import run_bass_kernel_spmd

    bf16 = _bf16()
    x = np.asarray(x, dtype=np.float32)
    qkv_w = np.asarray(qkv_w, dtype=np.float32)
    qkv_b = np.asarray(qkv_b, dtype=np.float32)
    proj_w = np.asarray(proj_w, dtype=np.float32)
    proj_b = np.asarray(proj_b, dtype=np.float32)

    nc = _get_nc()
    keepT = _get_keepT()

    qkvw_t = np.ascontiguousarray(qkv_w.T).astype(bf16)  # [C, 3C]
    projw_t = np.ascontiguousarray(proj_w.T).astype(bf16)  # [C, C]
    qkb = np.ascontiguousarray(qkv_b[:1024].reshape(8, 128).T)  # [128, 8]
    pjb = np.ascontiguousarray(proj_b.reshape(4, 128).T)  # [128, 4]
    bv = np.zeros((8, 66), np.float32)
    bv[:, :64] = qkv_b[1024:].reshape(8, 64)
    bv = bv.reshape(1, 8 * 66)

    in_maps = []
    for b in range(B):
        in_maps.append(
            {
                "xT": np.ascontiguousarray(x[b].T).astype(bf16),
                "qkvw": qkvw_t,
                "projw": projw_t,
                "keepT": keepT[b],
                "qkb": qkb,
                "bv": bv,
                "pjb": pjb,
            }
        )

    global _last_in_maps
    _last_in_maps = in_maps
    res = run_bass_kernel_spmd(nc, in_maps, core_ids=list(range(8)))
    out = np.stack([res.results[b]["out"].T for b in range(B)])  # [B, N, C]
    return out.astype(np.float32)


_last_in_maps = None
